# revision 1
# baseline (speedup 1.0000x reference)
"""Sparse multi-head attention (per-head strided K/V subsampling) for trn2.

Problem (hardcoded):
  query/key/value: (2048, 8, 512) f32, attn_mask: (8, 2048) bool,
  proj_w: (512, 512), proj_b: (512,).
  Per head h (8 heads, head_dim 64) with stride ksz in [4,4,2,2,1,1,1,1]:
    scores = q_h @ k_h[::ksz].T * 0.125, masked softmax over subsampled keys,
    o_h = softmax @ v_h[::ksz].
  Reference then does a RAW reshape (B,T,D)->(T,B,D) per head before concat +
  out-projection.  That reshape is a pure row permutation of the flattened
  (B*T, 512) matrix, so computing per-(batch,head) attention in (t, d) layout,
  concatenating per batch, projecting, stacking batches, and reshaping
  (B*T, 512) -> (T, B, 512) reproduces it exactly.

Sharding: batch-parallel, one batch element per NeuronCore (8 cores).

Device kernel layout choices (per core):
  - scores are computed TRANSPOSED (s on partitions, t on free) so that the
    P@V matmul can consume exp(scores) directly with V in natural layout.
    Q^T and K^T (head-dim on partitions) are prepared on the host so no
    on-device transposes are needed.
  - the attention mask becomes a per-partition bias (0 / -30000) fused into
    the ACT exp instruction: exp(0.125 * qk + bias).
  - V is augmented with a ones column: one matmul accumulates both unnorm
    output (64 rows) and the softmax denominator (row 64).
  - normalization: DVE reciprocal of the denominator row, gpsimd
    partition-broadcast, DVE multiply.
  - out-projection from the transposed unnormalized output with proj_w.T
    (host-prepared), bias added via a K=1 ones matmul.
"""

import numpy as np

import concourse.bass as bass
import concourse.tile as tile
from concourse import bacc, mybir
from concourse.bass_utils import run_bass_kernel_spmd

T = 2048
B = 8
E = 512
H = 8
D = 64
KS = [4, 4, 2, 2, 1, 1, 1, 1]
SCALE = 0.125
MASK_BIAS = -30000.0
P = 128
F32 = mybir.dt.float32
F32R = mybir.dt.float32r

SC = [T // k for k in KS]          # keys per head: 512,512,1024,1024,2048x4
NCH = [s // P for s in SC]         # s-chunks of 128: 4,4,8,8,16,16,16,16
THALF = 1024                       # t span per inner block (2 psum banks)


def _mm_dtype(ap):
    return ap


def build_program():
    nc = bacc.Bacc("TRN2", target_bir_lowering=False, debug=False, num_devices=B)

    qT = nc.dram_tensor("qT", [E, T], F32R, kind="ExternalInput")
    k4T = nc.dram_tensor("k4T", [128, 512], F32R, kind="ExternalInput")
    k2T = nc.dram_tensor("k2T", [128, 1024], F32R, kind="ExternalInput")
    k1T = nc.dram_tensor("k1T", [256, 2048], F32R, kind="ExternalInput")
    va4 = nc.dram_tensor("va4", [512, 130], F32R, kind="ExternalInput")
    va2 = nc.dram_tensor("va2", [1024, 130], F32R, kind="ExternalInput")
    va1 = nc.dram_tensor("va1", [2048, 260], F32R, kind="ExternalInput")
    mb4 = nc.dram_tensor("mb4", [128, 4], F32, kind="ExternalInput")
    mb2 = nc.dram_tensor("mb2", [128, 8], F32, kind="ExternalInput")
    mb1 = nc.dram_tensor("mb1", [128, 16], F32, kind="ExternalInput")
    wT = nc.dram_tensor("wT", [E, E], F32R, kind="ExternalInput")
    pb = nc.dram_tensor("pb", [1, E], F32R, kind="ExternalInput")
    onesd = nc.dram_tensor("onesd", [1, P], F32R, kind="ExternalInput")
    out = nc.dram_tensor("out", [T, E], F32, kind="ExternalOutput")

    with tile.TileContext(nc) as tc:
        with (
            tc.tile_pool(name="const", bufs=1) as cpool,
            tc.tile_pool(name="exp", bufs=4) as epool,
            tc.tile_pool(name="norm", bufs=2) as npool,
            tc.tile_pool(name="outsb", bufs=4) as opool,
        ):
            # ---- persistent SBUF loads ----
            qT_sb = []
            for p in range(4):
                t_ = cpool.tile([P, T], F32R, name=f"qT{p}", tag=f"qT{p}")
                nc.sync.dma_start(t_[:], qT.ap()[p * P:(p + 1) * P, :])
                qT_sb.append(t_)

            k4_sb = cpool.tile([P, 512], F32R, name="k4", tag="k4")
            nc.sync.dma_start(k4_sb[:], k4T.ap())
            k2_sb = cpool.tile([P, 1024], F32R, name="k2", tag="k2")
            nc.sync.dma_start(k2_sb[:], k2T.ap())
            k1_sb0 = cpool.tile([P, 2048], F32R, name="k1a", tag="k1a")
            nc.sync.dma_start(k1_sb0[:], k1T.ap()[0:128, :])
            k1_sb1 = cpool.tile([P, 2048], F32R, name="k1b", tag="k1b")
            nc.sync.dma_start(k1_sb1[:], k1T.ap()[128:256, :])

            va4_sb = cpool.tile([P, 4 * 130], F32R, name="va4s", tag="va4s")
            for j in range(4):
                nc.sync.dma_start(va4_sb[:, j * 130:(j + 1) * 130],
                                  va4.ap()[j * P:(j + 1) * P, :])
            va2_sb = cpool.tile([P, 8 * 130], F32R, name="va2s", tag="va2s")
            for j in range(8):
                nc.sync.dma_start(va2_sb[:, j * 130:(j + 1) * 130],
                                  va2.ap()[j * P:(j + 1) * P, :])
            va1_sb = cpool.tile([P, 16 * 260], F32R, name="va1s", tag="va1s")
            for j in range(16):
                nc.sync.dma_start(va1_sb[:, j * 260:(j + 1) * 260],
                                  va1.ap()[j * P:(j + 1) * P, :])

            mb4_sb = cpool.tile([P, 4], F32, name="mb4s", tag="mb4s")
            nc.sync.dma_start(mb4_sb[:], mb4.ap())
            mb2_sb = cpool.tile([P, 8], F32, name="mb2s", tag="mb2s")
            nc.sync.dma_start(mb2_sb[:], mb2.ap())
            mb1_sb = cpool.tile([P, 16], F32, name="mb1s", tag="mb1s")
            nc.sync.dma_start(mb1_sb[:], mb1.ap())

            wT_sb = []
            for i in range(4):
                t_ = cpool.tile([P, E], F32R, name=f"wT{i}", tag=f"wT{i}")
                nc.sync.dma_start(t_[:], wT.ap()[i * P:(i + 1) * P, :])
                wT_sb.append(t_)
            pb_sb = cpool.tile([1, E], F32R, name="pbs", tag="pbs")
            nc.sync.dma_start(pb_sb[:], pb.ap())
            ones_sb = cpool.tile([1, P], F32R, name="ones", tag="ones")
            nc.sync.dma_start(ones_sb[:], onesd.ap())

            # per-head views
            def kT_h(h):
                base = [k4_sb, k4_sb, k2_sb, k2_sb, k1_sb0, k1_sb0, k1_sb1,
                        k1_sb1][h]
                r0 = (h % 2) * 64
                return base[r0:r0 + 64, :]

            def qT_h(h):
                return qT_sb[h // 2][(h % 2) * 64:(h % 2) * 64 + 64, :]

            def va_h(h, j):
                if h < 2:
                    return va4_sb[:, j * 130 + h * 65: j * 130 + h * 65 + 65]
                if h < 4:
                    return va2_sb[:, j * 130 + (h - 2) * 65:
                                  j * 130 + (h - 2) * 65 + 65]
                return va1_sb[:, j * 260 + (h - 4) * 65:
                              j * 260 + (h - 4) * 65 + 65]

            def mb_h(h):
                return [mb4_sb, mb4_sb, mb2_sb, mb2_sb, mb1_sb, mb1_sb,
                        mb1_sb, mb1_sb][h]

            # transposed (unnormalized->normalized) head outputs, feeding proj
            oT_sb = []
            for p in range(4):
                t_ = cpool.tile([P, T], F32R, name=f"oT{p}", tag=f"oT{p}")
                oT_sb.append(t_)

            # ---- attention main loop ----
            with tc.tile_pool(name="psA", bufs=1, space="PSUM") as pspool:
                for h in range(H):
                    for th in range(T // THALF):
                        t0 = th * THALF
                        po = pspool.tile([65, THALF], F32, name="po",
                                         tag="po", bufs=2)
                        for j in range(NCH[h]):
                            ps = pspool.tile([P, THALF], F32, name="ps",
                                             tag="ps", bufs=2)
                            for tq in range(THALF // 512):
                                nc.tensor.matmul(
                                    ps[:, tq * 512:(tq + 1) * 512],
                                    lhsT=_mm_dtype(
                                        kT_h(h)[:, j * P:(j + 1) * P]),
                                    rhs=_mm_dtype(
                                        qT_h(h)[:, t0 + tq * 512:
                                                t0 + (tq + 1) * 512]),
                                    start=True, stop=True)
                            ex = epool.tile([P, THALF], F32R, name="ex",
                                            tag="ex")
                            nc.scalar.activation(
                                ex[:], ps[:],
                                mybir.ActivationFunctionType.Exp,
                                bias=mb_h(h)[:, j:j + 1], scale=SCALE)
                            for tq in range(THALF // 512):
                                nc.tensor.matmul(
                                    po[:, tq * 512:(tq + 1) * 512],
                                    lhsT=_mm_dtype(va_h(h, j)),
                                    rhs=_mm_dtype(
                                        ex[:, tq * 512:(tq + 1) * 512]),
                                    start=(j == 0), stop=(j == NCH[h] - 1))
                        # normalize: rows 0:64 are o^T, row 64 is denom
                        rec = npool.tile([1, THALF], F32, name="rec",
                                         tag="rec")
                        nc.vector.reciprocal(rec[:], po[64:65, :])
                        rbc = npool.tile([64, THALF], F32, name="rbc",
                                         tag="rbc")
                        nc.gpsimd.partition_broadcast(rbc[:], rec[:])
                        r0 = (h % 2) * 64
                        nc.vector.tensor_mul(
                            oT_sb[h // 2][r0:r0 + 64, t0:t0 + THALF],
                            po[0:64, :], rbc[:])

            # ---- output projection ----
            with tc.tile_pool(name="psP", bufs=2, space="PSUM") as pppool:
                for tq in range(T // P):
                    pp = pppool.tile([P, E], F32, name="pp", tag="pp")
                    for i in range(4):
                        nc.tensor.matmul(
                            pp[:],
                            lhsT=_mm_dtype(oT_sb[i][:, tq * P:(tq + 1) * P]),
                            rhs=_mm_dtype(wT_sb[i][:]),
                            start=(i == 0), stop=False)
                    nc.tensor.matmul(
                        pp[:], lhsT=_mm_dtype(ones_sb[:, 0:P]),
                        rhs=_mm_dtype(pb_sb[:]), start=False, stop=True)
                    ot = opool.tile([P, E], F32, name="ot", tag="ot")
                    nc.vector.tensor_copy(ot[:], pp[:])
                    nc.sync.dma_start(out.ap()[tq * P:(tq + 1) * P, :], ot[:])

    nc.compile()
    return nc


_PROGRAM = None


def _get_program():
    global _PROGRAM
    if _PROGRAM is None:
        _PROGRAM = build_program()
    return _PROGRAM


def _prep_core_inputs(query, key, value, attn_mask, wT, pb):
    ins = []
    ones = np.ones((1,), dtype=np.float32)
    for b in range(B):
        qb = np.ascontiguousarray(query[:, b, :].T)          # (512, 2048)
        kb = key[:, b, :]
        vb = value[:, b, :]
        k4T = np.ascontiguousarray(kb[::4, 0:128].T)          # (128, 512)
        k2T = np.ascontiguousarray(kb[::2, 128:256].T)        # (128, 1024)
        k1T = np.ascontiguousarray(kb[:, 256:512].T)          # (256, 2048)

        def build_va(v_sub, heads):
            n = v_sub.shape[0]
            va = np.empty((n, 65 * len(heads)), dtype=np.float32)
            for i, h in enumerate(heads):
                va[:, i * 65:i * 65 + 64] = v_sub[:, h * 64:(h + 1) * 64]
                va[:, i * 65 + 64] = 1.0
            return va

        va4 = build_va(vb[::4], [0, 1])
        va2 = build_va(vb[::2], [2, 3])
        va1 = build_va(vb, [4, 5, 6, 7])

        mb = attn_mask[b]

        def build_mb(msub):
            bias = np.where(msub, np.float32(MASK_BIAS), np.float32(0.0))
            nch = bias.shape[0] // P
            return np.ascontiguousarray(bias.reshape(nch, P).T)

        ins.append({
            "qT": qb, "k4T": k4T, "k2T": k2T, "k1T": k1T,
            "va4": va4, "va2": va2, "va1": va1,
            "mb4": build_mb(mb[::4]), "mb2": build_mb(mb[::2]),
            "mb1": build_mb(mb),
            "wT": wT, "pb": pb,
            "onesd": np.ones((1, 128), dtype=np.float32),
        })
    return ins


def kernel(query, key, value, attn_mask, proj_w, proj_b, _trace=False,
           **run_kwargs):
    query = np.asarray(query, dtype=np.float32)
    key = np.asarray(key, dtype=np.float32)
    value = np.asarray(value, dtype=np.float32)
    attn_mask = np.asarray(attn_mask).astype(bool)
    wT = np.ascontiguousarray(np.asarray(proj_w, dtype=np.float32).T)
    pb = np.ascontiguousarray(
        np.asarray(proj_b, dtype=np.float32).reshape(1, E))

    nc = _get_program()
    ins = _prep_core_inputs(query, key, value, attn_mask, wT, pb)
    res = run_bass_kernel_spmd(nc, ins, list(range(B)), trace=_trace,
                               **run_kwargs)
    outs = [np.asarray(res.results[b]["out"]) for b in range(B)]
    full = np.concatenate(outs, axis=0)          # (B*T, E), b-major rows
    result = full.reshape(T, B, E)
    if _trace:
        return result, res
    return result



# revision 3
# speedup vs baseline: 2.1874x; 2.1874x over previous
"""Sparse multi-head attention (per-head strided K/V subsampling) for trn2.

Problem (hardcoded):
  query/key/value: (2048, 8, 512) f32, attn_mask: (8, 2048) bool,
  proj_w: (512, 512), proj_b: (512,).
  Per head h (8 heads, head_dim 64) with stride ksz in [4,4,2,2,1,1,1,1]:
    scores = q_h @ k_h[::ksz].T * 0.125, masked softmax over subsampled keys,
    o_h = softmax @ v_h[::ksz].
  Reference then does a RAW reshape (B,T,D)->(T,B,D) per head before concat +
  out-projection.  That reshape is a pure row permutation of the flattened
  (B*T, 512) matrix, so computing per-(batch,head) attention in (t, d) layout,
  concatenating per batch, projecting, stacking batches, and reshaping
  (B*T, 512) -> (T, B, 512) reproduces it exactly.

Sharding: batch-parallel, one batch element per NeuronCore (8 cores).

v2 design (vs the f32r baseline):
  - mask-gather on the host: masked keys (True in attn_mask) contribute
    exactly zero to the softmax, so only the unmasked subsampled keys are
    shipped to the device (~50% of them for this mask distribution). This
    halves the scores matmuls, the exp (ACT) work, and the PV matmuls.
    Padding rows are all-zero INCLUDING the ones-column of the V-augmented
    matrix, so pad lanes contribute exp(0)=1 to nothing (their V and their
    denominator entries are 0).
  - all matmul operands in fp16 (f32r streams ~3x slower per row on HW).
  - scores computed transposed (s on partitions, t free) so PV consumes
    exp(scores) directly; V augmented with a ones column accumulates the
    softmax denominator in the same matmul (row 64 of po).
  - normalization: DVE reciprocal_approx_fast (the accurate `reciprocal`
    costs 6.5us per row on a 1-partition AP), gpsimd partition-broadcast,
    DVE multiply into fp16 oT.
  - out-projection from transposed normalized head outputs with host-side
    proj_w.T; bias folded into the DVE psum->sbuf copy (tensor_add with a
    pre-broadcast bias tile).
  - loop order: t-half outer, heads inner; t-half-0 projection chunks are
    interleaved into t-half-1's head loop to hide the projection tail.
"""

import numpy as np

import concourse.bass as bass
import concourse.tile as tile
from concourse import bacc, mybir
from concourse.bass_utils import run_bass_kernel_spmd

T = 2048
B = 8
E = 512
H = 8
D = 64
KS = [4, 4, 2, 2, 1, 1, 1, 1]
SCALE = 0.125
P = 128
THALF = 1024
F32 = mybir.dt.float32
F16 = mybir.dt.float16


def build_program(nch4, nch2, nch1):
    N4, N2, N1 = nch4 * P, nch2 * P, nch1 * P
    nc = bacc.Bacc("TRN2", target_bir_lowering=False, debug=False, num_devices=B)

    qT = nc.dram_tensor("qT", [E, T], F16, kind="ExternalInput")
    k4T = nc.dram_tensor("k4T", [P, N4], F16, kind="ExternalInput")
    k2T = nc.dram_tensor("k2T", [P, N2], F16, kind="ExternalInput")
    k1Ta = nc.dram_tensor("k1Ta", [P, N1], F16, kind="ExternalInput")
    k1Tb = nc.dram_tensor("k1Tb", [P, N1], F16, kind="ExternalInput")
    va4 = nc.dram_tensor("va4", [P, nch4 * 130], F16, kind="ExternalInput")
    va2 = nc.dram_tensor("va2", [P, nch2 * 130], F16, kind="ExternalInput")
    va1 = nc.dram_tensor("va1", [P, nch1 * 260], F16, kind="ExternalInput")
    wT = nc.dram_tensor("wT", [E, E], F16, kind="ExternalInput")
    pb = nc.dram_tensor("pb", [1, E], F32, kind="ExternalInput")
    out = nc.dram_tensor("out", [T, E], F32, kind="ExternalOutput")

    NCH = [nch4, nch4, nch2, nch2, nch1, nch1, nch1, nch1]

    with tile.TileContext(nc) as tc:
        with (
            tc.tile_pool(name="const", bufs=1) as cpool,
            tc.tile_pool(name="exp", bufs=4) as epool,
            tc.tile_pool(name="norm", bufs=2) as npool,
            tc.tile_pool(name="outsb", bufs=4) as opool,
            tc.tile_pool(name="psA", bufs=1, space="PSUM") as pspool,
        ):
            # ---- persistent SBUF loads (ordered by first use) ----
            k4_sb = cpool.tile([P, N4], F16, name="k4", tag="k4")
            nc.sync.dma_start(k4_sb[:], k4T.ap())
            qT_sb = []
            for p_ in range(4):
                t_ = cpool.tile([P, T], F16, name=f"qT{p_}", tag=f"qT{p_}")
                qT_sb.append(t_)
            nc.sync.dma_start(qT_sb[0][:], qT.ap()[0:P, :])
            va4_sb = cpool.tile([P, nch4 * 130], F16, name="va4s", tag="va4s")
            nc.sync.dma_start(va4_sb[:], va4.ap())
            k2_sb = cpool.tile([P, N2], F16, name="k2", tag="k2")
            nc.sync.dma_start(k2_sb[:], k2T.ap())
            nc.sync.dma_start(qT_sb[1][:], qT.ap()[P:2 * P, :])
            va2_sb = cpool.tile([P, nch2 * 130], F16, name="va2s", tag="va2s")
            nc.sync.dma_start(va2_sb[:], va2.ap())
            k1a_sb = cpool.tile([P, N1], F16, name="k1a", tag="k1a")
            nc.sync.dma_start(k1a_sb[:], k1Ta.ap())
            nc.sync.dma_start(qT_sb[2][:], qT.ap()[2 * P:3 * P, :])
            k1b_sb = cpool.tile([P, N1], F16, name="k1b", tag="k1b")
            nc.sync.dma_start(k1b_sb[:], k1Tb.ap())
            nc.sync.dma_start(qT_sb[3][:], qT.ap()[3 * P:4 * P, :])
            va1_sb = cpool.tile([P, nch1 * 260], F16, name="va1s", tag="va1s")
            nc.sync.dma_start(va1_sb[:], va1.ap())
            wT_sb = []
            for i in range(4):
                t_ = cpool.tile([P, E], F16, name=f"wT{i}", tag=f"wT{i}")
                nc.sync.dma_start(t_[:], wT.ap()[i * P:(i + 1) * P, :])
                wT_sb.append(t_)
            pb_sb = cpool.tile([1, E], F32, name="pbs", tag="pbs")
            nc.sync.dma_start(pb_sb[:], pb.ap())
            pbb_sb = cpool.tile([P, E], F32, name="pbb", tag="pbb")
            nc.gpsimd.partition_broadcast(pbb_sb[:], pb_sb[:])

            # per-head views
            def kT_h(h):
                base = [k4_sb, k4_sb, k2_sb, k2_sb, k1a_sb, k1a_sb, k1b_sb,
                        k1b_sb][h]
                r0 = (h % 2) * 64
                return base[r0:r0 + 64, :]

            def qT_h(h):
                return qT_sb[h // 2][(h % 2) * 64:(h % 2) * 64 + 64, :]

            def va_h(h, j):
                if h < 2:
                    return va4_sb[:, j * 130 + h * 65: j * 130 + h * 65 + 65]
                if h < 4:
                    return va2_sb[:, j * 130 + (h - 2) * 65:
                                  j * 130 + (h - 2) * 65 + 65]
                return va1_sb[:, j * 260 + (h - 4) * 65:
                              j * 260 + (h - 4) * 65 + 65]

            # transposed normalized head outputs (fp16), feeding proj
            oT_sb = []
            for p_ in range(4):
                t_ = cpool.tile([P, T], F16, name=f"oT{p_}", tag=f"oT{p_}")
                oT_sb.append(t_)

            def proj_chunk(tq):
                pp_full = pspool.tile([P, THALF], F32, name="pp", tag="ps",
                                      bufs=2)
                pp = pp_full[:, 0:E]
                for i in range(4):
                    nc.tensor.matmul(
                        pp, lhsT=oT_sb[i][:, tq * P:(tq + 1) * P],
                        rhs=wT_sb[i][:], start=(i == 0), stop=(i == 3))
                ot = opool.tile([P, E], F32, name="ot", tag="ot")
                nc.vector.tensor_add(ot[:], pp, pbb_sb[:])
                nc.sync.dma_start(out.ap()[tq * P:(tq + 1) * P, :], ot[:])

            # ---- attention main loop ----
            for th in range(2):
                t0 = th * THALF
                for h in range(H):
                    nchh = NCH[h]
                    po = pspool.tile([P, THALF], F32, name="po", tag="po",
                                     bufs=2)
                    exs = []
                    for j in range(nchh):
                        ps = pspool.tile([P, THALF], F32, name="ps", tag="ps",
                                         bufs=2)
                        for tq in range(2):
                            nc.tensor.matmul(
                                ps[:, tq * 512:(tq + 1) * 512],
                                lhsT=kT_h(h)[:, j * P:(j + 1) * P],
                                rhs=qT_h(h)[:, t0 + tq * 512:
                                            t0 + (tq + 1) * 512],
                                start=True, stop=True)
                        ex = epool.tile([P, THALF], F16, name="ex", tag="ex")
                        nc.scalar.activation(
                            ex[:], ps[:], mybir.ActivationFunctionType.Exp,
                            bias=0.0, scale=SCALE)
                        exs.append(ex)
                        if j >= 1:
                            for tq in range(2):
                                nc.tensor.matmul(
                                    po[0:65, tq * 512:(tq + 1) * 512],
                                    lhsT=va_h(h, j - 1),
                                    rhs=exs[j - 1][:, tq * 512:(tq + 1) * 512],
                                    start=(j - 1 == 0), stop=False)
                    j = nchh - 1
                    for tq in range(2):
                        nc.tensor.matmul(
                            po[0:65, tq * 512:(tq + 1) * 512],
                            lhsT=va_h(h, j),
                            rhs=exs[j][:, tq * 512:(tq + 1) * 512],
                            start=(j == 0), stop=True)
                    # normalize: rows 0:64 are o^T, row 64 is denom.
                    # custom-DVE ops can't read PSUM correctly; stage the
                    # denominator row through SBUF first.
                    den = npool.tile([1, THALF], F32, name="den", tag="den")
                    nc.vector.tensor_copy(den[:], po[64:65, :])
                    rec = npool.tile([1, THALF], F32, name="rec", tag="rec")
                    nc.vector.reciprocal_approx_fast(rec[:], den[:])
                    rbc = npool.tile([64, THALF], F32, name="rbc", tag="rbc")
                    nc.gpsimd.partition_broadcast(rbc[:], rec[:])
                    r0 = (h % 2) * 64
                    nc.vector.tensor_mul(
                        oT_sb[h // 2][r0:r0 + 64, t0:t0 + THALF],
                        po[0:64, :], rbc[:])
                    if th == 1:
                        proj_chunk(h)
            for tq in range(8, 16):
                proj_chunk(tq)

    nc.compile()
    return nc


_PROGRAMS = {}


def _get_program(nch4, nch2, nch1):
    key = (nch4, nch2, nch1)
    if key not in _PROGRAMS:
        _PROGRAMS[key] = build_program(*key)
    return _PROGRAMS[key]


def _prep_core_inputs(query, key, value, mask, wT, pb, keeps, nchs):
    nch4, nch2, nch1 = nchs
    N4, N2, N1 = nch4 * P, nch2 * P, nch1 * P
    ins = []
    for b in range(B):
        qb = np.ascontiguousarray(query[:, b, :].T).astype(np.float16)

        def build_k(sub, idx, c0, c1, N):
            # sub: (Ssub, 512) strided keys; gather idx rows, heads c0:c1
            z = np.zeros((P, N), dtype=np.float16)
            g = sub[idx]
            n = len(idx)
            z[:, 0:n] = g[:, c0:c1].T.astype(np.float16)
            return z

        def build_va(sub, idx, heads, W, nch):
            g = sub[idx]
            n = len(idx)
            z = np.zeros((P, nch * W), dtype=np.float16)
            for j in range(nch):
                seg = g[j * P:(j + 1) * P]
                m = seg.shape[0]
                if m == 0:
                    break
                for i, h in enumerate(heads):
                    z[0:m, j * W + i * 65: j * W + i * 65 + 64] = \
                        seg[:, h * 64:(h + 1) * 64].astype(np.float16)
                    z[0:m, j * W + i * 65 + 64] = 1.0
            return z

        kb, vb = key[:, b, :], value[:, b, :]
        i4, i2, i1 = keeps[4][b], keeps[2][b], keeps[1][b]
        ins.append({
            "qT": qb,
            "k4T": build_k(kb[::4], i4, 0, 128, N4),
            "k2T": build_k(kb[::2], i2, 128, 256, N2),
            "k1Ta": build_k(kb, i1, 256, 384, N1),
            "k1Tb": build_k(kb, i1, 384, 512, N1),
            "va4": build_va(vb[::4], i4, [0, 1], 130, nch4),
            "va2": build_va(vb[::2], i2, [2, 3], 130, nch2),
            "va1": build_va(vb, i1, [4, 5, 6, 7], 260, nch1),
            "wT": wT, "pb": pb,
        })
    return ins


def kernel(query, key, value, attn_mask, proj_w, proj_b, _trace=False,
           **run_kwargs):
    query = np.asarray(query, dtype=np.float32)
    key = np.asarray(key, dtype=np.float32)
    value = np.asarray(value, dtype=np.float32)
    mask = np.asarray(attn_mask).astype(bool)
    wT = np.ascontiguousarray(
        np.asarray(proj_w, dtype=np.float32).T).astype(np.float16)
    pb = np.ascontiguousarray(
        np.asarray(proj_b, dtype=np.float32).reshape(1, E))

    keeps = {ks: [np.flatnonzero(~mask[b, ::ks]) for b in range(B)]
             for ks in (4, 2, 1)}
    nchs = tuple(
        max(1, -(-max(len(keeps[ks][b]) for b in range(B)) // P))
        for ks in (4, 2, 1))

    nc = _get_program(*nchs)
    ins = _prep_core_inputs(query, key, value, mask, wT, pb, keeps, nchs)
    res = run_bass_kernel_spmd(nc, ins, list(range(B)), trace=_trace,
                               **run_kwargs)
    outs = [np.asarray(res.results[b]["out"]) for b in range(B)]
    full = np.concatenate(outs, axis=0)          # (B*T, E), b-major rows
    result = full.reshape(T, B, E)
    if _trace:
        return result, res
    return result


# revision 6
# speedup vs baseline: 2.5943x; 1.1860x over previous
"""Sparse multi-head attention (per-head strided K/V subsampling) for trn2.

Problem (hardcoded):
  query/key/value: (2048, 8, 512) f32, attn_mask: (8, 2048) bool,
  proj_w: (512, 512), proj_b: (512,).
  Per head h (8 heads, head_dim 64) with stride ksz in [4,4,2,2,1,1,1,1]:
    scores = q_h @ k_h[::ksz].T * 0.125, masked softmax over subsampled keys,
    o_h = softmax @ v_h[::ksz].
  Reference then does a RAW reshape (B,T,D)->(T,B,D) per head before concat +
  out-projection.  That reshape is a pure row permutation of the flattened
  (B*T, 512) matrix, so computing per-(batch,head) attention in (t, d) layout,
  concatenating per batch, projecting, stacking batches, and reshaping
  (B*T, 512) -> (T, B, 512) reproduces it exactly.

Sharding: batch-parallel, one batch element per NeuronCore (8 cores).

v2 design (vs the f32r baseline):
  - mask-gather on the host: masked keys (True in attn_mask) contribute
    exactly zero to the softmax, so only the unmasked subsampled keys are
    shipped to the device (~50% of them for this mask distribution). This
    halves the scores matmuls, the exp (ACT) work, and the PV matmuls.
    Padding rows are all-zero INCLUDING the ones-column of the V-augmented
    matrix, so pad lanes contribute exp(0)=1 to nothing (their V and their
    denominator entries are 0).
  - all matmul operands in fp16 (f32r streams ~3x slower per row on HW).
  - scores computed transposed (s on partitions, t free) so PV consumes
    exp(scores) directly; V augmented with a ones column accumulates the
    softmax denominator in the same matmul (row 64 of po).
  - normalization: DVE reciprocal_approx_fast (the accurate `reciprocal`
    costs 6.5us per row on a 1-partition AP), gpsimd partition-broadcast,
    DVE multiply into fp16 oT.
  - out-projection from transposed normalized head outputs with host-side
    proj_w.T; bias folded into the DVE psum->sbuf copy (tensor_add with a
    pre-broadcast bias tile).
  - loop order: t-half outer, heads inner; t-half-0 projection chunks are
    interleaved into t-half-1's head loop to hide the projection tail.
"""

import numpy as np

import concourse.bass as bass
import concourse.tile as tile
from concourse import bacc, mybir
from concourse.bass_utils import run_bass_kernel_spmd

T = 2048
B = 8
E = 512
H = 8
D = 64
KS = [4, 4, 2, 2, 1, 1, 1, 1]
SCALE = 0.125
P = 128
THALF = 1024
F32 = mybir.dt.float32
F16 = mybir.dt.float16


def build_program(nch4, nch2, nch1):
    N4, N2, N1 = nch4 * P, nch2 * P, nch1 * P
    nc = bacc.Bacc("TRN2", target_bir_lowering=False, debug=False, num_devices=B)

    qT = nc.dram_tensor("qT", [E, T], F16, kind="ExternalInput")
    k4T = nc.dram_tensor("k4T", [P, N4], F16, kind="ExternalInput")
    k2T = nc.dram_tensor("k2T", [P, N2], F16, kind="ExternalInput")
    k1Ta = nc.dram_tensor("k1Ta", [P, N1], F16, kind="ExternalInput")
    k1Tb = nc.dram_tensor("k1Tb", [P, N1], F16, kind="ExternalInput")
    va4 = nc.dram_tensor("va4", [P, nch4 * 130], F16, kind="ExternalInput")
    va2 = nc.dram_tensor("va2", [P, nch2 * 130], F16, kind="ExternalInput")
    va1 = nc.dram_tensor("va1", [P, nch1 * 260], F16, kind="ExternalInput")
    wT = nc.dram_tensor("wT", [E, E], F16, kind="ExternalInput")
    pb = nc.dram_tensor("pb", [1, E], F32, kind="ExternalInput")
    out = nc.dram_tensor("out", [T, E], F32, kind="ExternalOutput")

    NCH = [nch4, nch4, nch2, nch2, nch1, nch1, nch1, nch1]

    with tile.TileContext(nc) as tc:
        with (
            tc.tile_pool(name="const", bufs=1) as cpool,
            tc.tile_pool(name="exp", bufs=4) as epool,
            tc.tile_pool(name="norm", bufs=2) as npool,
            tc.tile_pool(name="outsb", bufs=4) as opool,
            tc.tile_pool(name="psA", bufs=1, space="PSUM") as pspool,
        ):
            # ---- persistent SBUF loads (ordered by first use) ----
            qT_sb = []
            for p_ in range(4):
                t_ = cpool.tile([P, T], F16, name=f"qT{p_}", tag=f"qT{p_}")
                qT_sb.append(t_)
            nc.sync.dma_start(qT_sb[0][:], qT.ap()[0:P, :])
            k4_sb = cpool.tile([P, N4], F16, name="k4", tag="k4")
            nc.sync.dma_start(k4_sb[:], k4T.ap())
            va4_sb = cpool.tile([P, nch4 * 130], F16, name="va4s", tag="va4s")
            nc.sync.dma_start(va4_sb[:], va4.ap())
            k2_sb = cpool.tile([P, N2], F16, name="k2", tag="k2")
            nc.sync.dma_start(k2_sb[:], k2T.ap())
            nc.sync.dma_start(qT_sb[1][:], qT.ap()[P:2 * P, :])
            va2_sb = cpool.tile([P, nch2 * 130], F16, name="va2s", tag="va2s")
            nc.sync.dma_start(va2_sb[:], va2.ap())
            k1a_sb = cpool.tile([P, N1], F16, name="k1a", tag="k1a")
            nc.sync.dma_start(k1a_sb[:], k1Ta.ap())
            nc.sync.dma_start(qT_sb[2][:], qT.ap()[2 * P:3 * P, :])
            k1b_sb = cpool.tile([P, N1], F16, name="k1b", tag="k1b")
            nc.sync.dma_start(k1b_sb[:], k1Tb.ap())
            nc.sync.dma_start(qT_sb[3][:], qT.ap()[3 * P:4 * P, :])
            va1_sb = cpool.tile([P, nch1 * 260], F16, name="va1s", tag="va1s")
            nc.sync.dma_start(va1_sb[:], va1.ap())
            wT_sb = []
            for i in range(4):
                t_ = cpool.tile([P, E], F16, name=f"wT{i}", tag=f"wT{i}")
                nc.sync.dma_start(t_[:], wT.ap()[i * P:(i + 1) * P, :])
                wT_sb.append(t_)
            pb_sb = cpool.tile([1, E], F32, name="pbs", tag="pbs")
            nc.sync.dma_start(pb_sb[:], pb.ap())
            pbb_sb = cpool.tile([P, E], F32, name="pbb", tag="pbb")
            nc.gpsimd.partition_broadcast(pbb_sb[:], pb_sb[:])

            # ---- PE warm-up burst ----
            # The HAM clock gate keeps the PE at 1.2 GHz until it sees ~3.4us
            # of sustained matmul activity; the steady-state loop has small
            # recurring gaps (LDWEIGHTS, ACT waits) that can fail the window.
            # Fill the initial DMA-wait dead time with back-to-back dummy
            # matmuls (no weight reloads) so the PE enters the main loop at
            # 2.4 GHz.
            wu_sb = cpool.tile([64, 512], F16, name="wu", tag="wu")
            nc.gpsimd.memset(wu_sb[:], 0.0)
            warm_ps = pspool.tile([P, THALF], F32, name="ps", tag="ps",
                                  bufs=2)
            for i in range(16):
                mmw = nc.tensor.matmul(
                    warm_ps[:, 0:512], lhsT=wu_sb[:, 0:P], rhs=wu_sb[:],
                    start=True, stop=True)
                if i > 0:
                    mmw.ins.ldweights = False

            # per-head views
            def kT_h(h):
                base = [k4_sb, k4_sb, k2_sb, k2_sb, k1a_sb, k1a_sb, k1b_sb,
                        k1b_sb][h]
                r0 = (h % 2) * 64
                return base[r0:r0 + 64, :]

            def qT_h(h):
                return qT_sb[h // 2][(h % 2) * 64:(h % 2) * 64 + 64, :]

            def va_h(h, j):
                if h < 2:
                    return va4_sb[:, j * 130 + h * 65: j * 130 + h * 65 + 65]
                if h < 4:
                    return va2_sb[:, j * 130 + (h - 2) * 65:
                                  j * 130 + (h - 2) * 65 + 65]
                return va1_sb[:, j * 260 + (h - 4) * 65:
                              j * 260 + (h - 4) * 65 + 65]

            # transposed normalized head outputs (fp16), feeding proj
            oT_sb = []
            for p_ in range(4):
                t_ = cpool.tile([P, T], F16, name=f"oT{p_}", tag=f"oT{p_}")
                oT_sb.append(t_)

            def proj_chunk(tq):
                pp_full = pspool.tile([P, THALF], F32, name="pp", tag="ps",
                                      bufs=2)
                pp = pp_full[:, 0:E]
                for i in range(4):
                    nc.tensor.matmul(
                        pp, lhsT=oT_sb[i][:, tq * P:(tq + 1) * P],
                        rhs=wT_sb[i][:], start=(i == 0), stop=(i == 3))
                ot = opool.tile([P, E], F32, name="ot", tag="ot")
                nc.vector.tensor_add(ot[:], pp, pbb_sb[:])
                nc.sync.dma_start(out.ap()[tq * P:(tq + 1) * P, :], ot[:])

            # ---- attention main loop ----
            for th in range(2):
                t0 = th * THALF
                for h in range(H):
                    nchh = NCH[h]
                    po = pspool.tile([P, THALF], F32, name="po", tag="po",
                                     bufs=2)
                    exs = []
                    def pv(j, last):
                        for tq in range(2):
                            mm = nc.tensor.matmul(
                                po[0:65, tq * 512:(tq + 1) * 512],
                                lhsT=va_h(h, j),
                                rhs=exs[j][:, tq * 512:(tq + 1) * 512],
                                start=(j == 0), stop=last)
                            if tq == 1:
                                mm.ins.ldweights = False

                    for j in range(nchh):
                        ps = pspool.tile([P, THALF], F32, name="ps", tag="ps",
                                         bufs=2)
                        for tq in range(2):
                            mm = nc.tensor.matmul(
                                ps[:, tq * 512:(tq + 1) * 512],
                                lhsT=kT_h(h)[:, j * P:(j + 1) * P],
                                rhs=qT_h(h)[:, t0 + tq * 512:
                                            t0 + (tq + 1) * 512],
                                start=True, stop=True)
                            if tq == 1:
                                mm.ins.ldweights = False
                        ex = epool.tile([P, THALF], F16, name="ex", tag="ex",
                                        bufs=6)
                        nc.scalar.activation(
                            ex[:], ps[:], mybir.ActivationFunctionType.Exp,
                            bias=0.0, scale=SCALE)
                        exs.append(ex)
                        if j >= 1:
                            pv(j - 1, last=False)
                    pv(nchh - 1, last=True)
                    # normalize: rows 0:64 are o^T, row 64 is denom.
                    # custom-DVE ops can't read PSUM correctly; stage the
                    # denominator row through SBUF first.
                    den = npool.tile([1, THALF], F32, name="den", tag="den")
                    nc.vector.tensor_copy(den[:], po[64:65, :])
                    rec = npool.tile([1, THALF], F32, name="rec", tag="rec")
                    nc.vector.reciprocal_approx_fast(rec[:], den[:])
                    rbc = npool.tile([64, THALF], F32, name="rbc", tag="rbc")
                    nc.gpsimd.partition_broadcast(rbc[:], rec[:])
                    r0 = (h % 2) * 64
                    nc.vector.tensor_mul(
                        oT_sb[h // 2][r0:r0 + 64, t0:t0 + THALF],
                        po[0:64, :], rbc[:])
                    if th == 1:
                        proj_chunk(h)
            for tq in range(8, 16):
                proj_chunk(tq)

    nc.compile()
    return nc


_PROGRAMS = {}


def _get_program(nch4, nch2, nch1):
    key = (nch4, nch2, nch1)
    if key not in _PROGRAMS:
        _PROGRAMS[key] = build_program(*key)
    return _PROGRAMS[key]


def _prep_core_inputs(query, key, value, mask, wT, pb, keeps, nchs):
    nch4, nch2, nch1 = nchs
    N4, N2, N1 = nch4 * P, nch2 * P, nch1 * P
    ins = []
    for b in range(B):
        qb = np.ascontiguousarray(query[:, b, :].T).astype(np.float16)

        def build_k(sub, idx, c0, c1, N):
            # sub: (Ssub, 512) strided keys; gather idx rows, heads c0:c1
            z = np.zeros((P, N), dtype=np.float16)
            g = sub[idx]
            n = len(idx)
            z[:, 0:n] = g[:, c0:c1].T.astype(np.float16)
            return z

        def build_va(sub, idx, heads, W, nch):
            g = sub[idx]
            n = len(idx)
            z = np.zeros((P, nch * W), dtype=np.float16)
            for j in range(nch):
                seg = g[j * P:(j + 1) * P]
                m = seg.shape[0]
                if m == 0:
                    break
                for i, h in enumerate(heads):
                    z[0:m, j * W + i * 65: j * W + i * 65 + 64] = \
                        seg[:, h * 64:(h + 1) * 64].astype(np.float16)
                    z[0:m, j * W + i * 65 + 64] = 1.0
            return z

        kb, vb = key[:, b, :], value[:, b, :]
        i4, i2, i1 = keeps[4][b], keeps[2][b], keeps[1][b]
        ins.append({
            "qT": qb,
            "k4T": build_k(kb[::4], i4, 0, 128, N4),
            "k2T": build_k(kb[::2], i2, 128, 256, N2),
            "k1Ta": build_k(kb, i1, 256, 384, N1),
            "k1Tb": build_k(kb, i1, 384, 512, N1),
            "va4": build_va(vb[::4], i4, [0, 1], 130, nch4),
            "va2": build_va(vb[::2], i2, [2, 3], 130, nch2),
            "va1": build_va(vb, i1, [4, 5, 6, 7], 260, nch1),
            "wT": wT, "pb": pb,
        })
    return ins


def kernel(query, key, value, attn_mask, proj_w, proj_b, _trace=False,
           **run_kwargs):
    query = np.asarray(query, dtype=np.float32)
    key = np.asarray(key, dtype=np.float32)
    value = np.asarray(value, dtype=np.float32)
    mask = np.asarray(attn_mask).astype(bool)
    wT = np.ascontiguousarray(
        np.asarray(proj_w, dtype=np.float32).T).astype(np.float16)
    pb = np.ascontiguousarray(
        np.asarray(proj_b, dtype=np.float32).reshape(1, E))

    keeps = {ks: [np.flatnonzero(~mask[b, ::ks]) for b in range(B)]
             for ks in (4, 2, 1)}
    nchs = tuple(
        max(1, -(-max(len(keeps[ks][b]) for b in range(B)) // P))
        for ks in (4, 2, 1))

    nc = _get_program(*nchs)
    ins = _prep_core_inputs(query, key, value, mask, wT, pb, keeps, nchs)
    res = run_bass_kernel_spmd(nc, ins, list(range(B)), trace=_trace,
                               **run_kwargs)
    outs = [np.asarray(res.results[b]["out"]) for b in range(B)]
    full = np.concatenate(outs, axis=0)          # (B*T, E), b-major rows
    result = full.reshape(T, B, E)
    if _trace:
        return result, res
    return result


# revision 9
# speedup vs baseline: 2.7993x; 1.0790x over previous
"""Sparse multi-head attention (per-head strided K/V subsampling) for trn2.

Problem (hardcoded):
  query/key/value: (2048, 8, 512) f32, attn_mask: (8, 2048) bool,
  proj_w: (512, 512), proj_b: (512,).
  Per head h (8 heads, head_dim 64) with stride ksz in [4,4,2,2,1,1,1,1]:
    scores = q_h @ k_h[::ksz].T * 0.125, masked softmax over subsampled keys,
    o_h = softmax @ v_h[::ksz].
  Reference then does a RAW reshape (B,T,D)->(T,B,D) per head before concat +
  out-projection.  That reshape is a pure row permutation of the flattened
  (B*T, 512) matrix, so computing per-(batch,head) attention in (t, d) layout,
  concatenating per batch, projecting, stacking batches, and reshaping
  (B*T, 512) -> (T, B, 512) reproduces it exactly.

Sharding: batch-parallel, one batch element per NeuronCore (8 cores).

v2 design (vs the f32r baseline):
  - mask-gather on the host: masked keys (True in attn_mask) contribute
    exactly zero to the softmax, so only the unmasked subsampled keys are
    shipped to the device (~50% of them for this mask distribution). This
    halves the scores matmuls, the exp (ACT) work, and the PV matmuls.
    Padding rows are all-zero INCLUDING the ones-column of the V-augmented
    matrix, so pad lanes contribute exp(0)=1 to nothing (their V and their
    denominator entries are 0).
  - all matmul operands in fp16 (f32r streams ~3x slower per row on HW).
  - scores computed transposed (s on partitions, t free) so PV consumes
    exp(scores) directly; V augmented with a ones column accumulates the
    softmax denominator in the same matmul (row 64 of po).
  - normalization: DVE reciprocal_approx_fast (the accurate `reciprocal`
    costs 6.5us per row on a 1-partition AP), gpsimd partition-broadcast,
    DVE multiply into fp16 oT.
  - out-projection from transposed normalized head outputs with host-side
    proj_w.T; bias folded into the DVE psum->sbuf copy (tensor_add with a
    pre-broadcast bias tile).
  - loop order: t-half outer, heads inner; t-half-0 projection chunks are
    interleaved into t-half-1's head loop to hide the projection tail.
"""

import numpy as np

import concourse.bass as bass
import concourse.tile as tile
from concourse import bacc, mybir
from concourse.bass_utils import run_bass_kernel_spmd

T = 2048
B = 8
E = 512
H = 8
D = 64
KS = [4, 4, 2, 2, 1, 1, 1, 1]
SCALE = 0.125
P = 128
THALF = 1024
F32 = mybir.dt.float32
F16 = mybir.dt.float16


def build_program(nch4, nch2, nch1):
    N4, N2, N1 = nch4 * P, nch2 * P, nch1 * P
    nc = bacc.Bacc("TRN2", target_bir_lowering=False, debug=False, num_devices=B)

    qT = nc.dram_tensor("qT", [E, T], F16, kind="ExternalInput")
    k4T = nc.dram_tensor("k4T", [P, N4], F16, kind="ExternalInput")
    k2T = nc.dram_tensor("k2T", [P, N2], F16, kind="ExternalInput")
    k1Ta = nc.dram_tensor("k1Ta", [P, N1], F16, kind="ExternalInput")
    k1Tb = nc.dram_tensor("k1Tb", [P, N1], F16, kind="ExternalInput")
    va4 = nc.dram_tensor("va4", [P, nch4 * 130], F16, kind="ExternalInput")
    va2 = nc.dram_tensor("va2", [P, nch2 * 130], F16, kind="ExternalInput")
    va1 = nc.dram_tensor("va1", [P, nch1 * 260], F16, kind="ExternalInput")
    wT = nc.dram_tensor("wT", [E, E], F16, kind="ExternalInput")
    pb = nc.dram_tensor("pb", [1, E], F32, kind="ExternalInput")
    out = nc.dram_tensor("out", [T, E], F32, kind="ExternalOutput")

    NCH = [nch4, nch4, nch2, nch2, nch1, nch1, nch1, nch1]

    with tile.TileContext(nc) as tc:
        with (
            tc.tile_pool(name="const", bufs=1) as cpool,
            tc.tile_pool(name="exp", bufs=4) as epool,
            tc.tile_pool(name="norm", bufs=2) as npool,
            tc.tile_pool(name="outsb", bufs=4) as opool,
            tc.tile_pool(name="psA", bufs=1, space="PSUM") as pspool,
        ):
            # ---- persistent SBUF loads (ordered by first use) ----
            qT_sb = []
            for p_ in range(4):
                t_ = cpool.tile([P, T], F16, name=f"qT{p_}", tag=f"qT{p_}")
                qT_sb.append(t_)
            nc.sync.dma_start(qT_sb[0][:], qT.ap()[0:P, :])
            k4_sb = cpool.tile([P, N4], F16, name="k4", tag="k4")
            nc.sync.dma_start(k4_sb[:], k4T.ap())
            va4_sb = cpool.tile([P, nch4 * 130], F16, name="va4s", tag="va4s")
            nc.sync.dma_start(va4_sb[:], va4.ap())
            k2_sb = cpool.tile([P, N2], F16, name="k2", tag="k2")
            nc.sync.dma_start(k2_sb[:], k2T.ap())
            nc.sync.dma_start(qT_sb[1][:], qT.ap()[P:2 * P, :])
            va2_sb = cpool.tile([P, nch2 * 130], F16, name="va2s", tag="va2s")
            nc.sync.dma_start(va2_sb[:], va2.ap())
            k1a_sb = cpool.tile([P, N1], F16, name="k1a", tag="k1a")
            nc.sync.dma_start(k1a_sb[:], k1Ta.ap())
            nc.sync.dma_start(qT_sb[2][:], qT.ap()[2 * P:3 * P, :])
            k1b_sb = cpool.tile([P, N1], F16, name="k1b", tag="k1b")
            nc.sync.dma_start(k1b_sb[:], k1Tb.ap())
            nc.sync.dma_start(qT_sb[3][:], qT.ap()[3 * P:4 * P, :])
            va1_sb = cpool.tile([P, nch1 * 260], F16, name="va1s", tag="va1s")
            nc.sync.dma_start(va1_sb[:], va1.ap())
            wT_sb = []
            for i in range(4):
                t_ = cpool.tile([P, E], F16, name=f"wT{i}", tag=f"wT{i}")
                nc.sync.dma_start(t_[:], wT.ap()[i * P:(i + 1) * P, :])
                wT_sb.append(t_)
            pb_sb = cpool.tile([1, E], F32, name="pbs", tag="pbs")
            nc.sync.dma_start(pb_sb[:], pb.ap())
            pbb_sb = cpool.tile([P, E], F32, name="pbb", tag="pbb")
            nc.gpsimd.partition_broadcast(pbb_sb[:], pb_sb[:])

            # ---- PE warm-up burst ----
            # The HAM clock gate keeps the PE at 1.2 GHz until it sees ~3.4us
            # of sustained matmul activity; the steady-state loop has small
            # recurring gaps (LDWEIGHTS, ACT waits) that can fail the window.
            # Fill the initial DMA-wait dead time with back-to-back dummy
            # matmuls (no weight reloads) so the PE enters the main loop at
            # 2.4 GHz.
            # Skinny stationary operand (4 cols): LDWEIGHTS ~40ns instead of
            # ~148ns, so the PE duty cycle in the burst stays ~90% and the
            # HAM activity window actually fires.
            wu_sb = cpool.tile([64, 512], F16, name="wu", tag="wu")
            nc.gpsimd.memset(wu_sb[:], 0.0)
            warm_ps = pspool.tile([P, THALF], F32, name="ps", tag="ps",
                                  bufs=2)
            for i in range(18):
                nc.tensor.matmul(
                    warm_ps[0:4, 0:512], lhsT=wu_sb[:, 0:4], rhs=wu_sb[:],
                    start=True, stop=True)

            # per-head views
            def kT_h(h):
                base = [k4_sb, k4_sb, k2_sb, k2_sb, k1a_sb, k1a_sb, k1b_sb,
                        k1b_sb][h]
                r0 = (h % 2) * 64
                return base[r0:r0 + 64, :]

            def qT_h(h):
                return qT_sb[h // 2][(h % 2) * 64:(h % 2) * 64 + 64, :]

            def va_h(h, j):
                if h < 2:
                    return va4_sb[:, j * 130 + h * 65: j * 130 + h * 65 + 65]
                if h < 4:
                    return va2_sb[:, j * 130 + (h - 2) * 65:
                                  j * 130 + (h - 2) * 65 + 65]
                return va1_sb[:, j * 260 + (h - 4) * 65:
                              j * 260 + (h - 4) * 65 + 65]

            # transposed normalized head outputs (fp16), feeding proj
            oT_sb = []
            for p_ in range(4):
                t_ = cpool.tile([P, T], F16, name=f"oT{p_}", tag=f"oT{p_}")
                oT_sb.append(t_)

            def proj_chunk(tq):
                pp_full = pspool.tile([P, THALF], F32, name="pp", tag="ps",
                                      bufs=2)
                pp = pp_full[:, 0:E]
                for i in range(4):
                    nc.tensor.matmul(
                        pp, lhsT=oT_sb[i][:, tq * P:(tq + 1) * P],
                        rhs=wT_sb[i][:], start=(i == 0), stop=(i == 3))
                ot = opool.tile([P, E], F32, name="ot", tag="ot")
                nc.vector.tensor_add(ot[:], pp, pbb_sb[:])
                nc.sync.dma_start(out.ap()[tq * P:(tq + 1) * P, :], ot[:])

            # ---- attention main loop ----
            for th in range(2):
                t0 = th * THALF
                for h in range(H):
                    nchh = NCH[h]
                    po = pspool.tile([P, THALF], F32, name="po", tag="po",
                                     bufs=2)
                    exs = []
                    def pv(j, last):
                        for tq in range(2):
                            mm = nc.tensor.matmul(
                                po[0:65, tq * 512:(tq + 1) * 512],
                                lhsT=va_h(h, j),
                                rhs=exs[j][:, tq * 512:(tq + 1) * 512],
                                start=(j == 0), stop=last)
                            if tq == 1:
                                mm.ins.ldweights = False

                    for j in range(nchh):
                        ps = pspool.tile([P, THALF], F32, name="ps", tag="ps",
                                         bufs=2)
                        for tq in range(2):
                            mm = nc.tensor.matmul(
                                ps[:, tq * 512:(tq + 1) * 512],
                                lhsT=kT_h(h)[:, j * P:(j + 1) * P],
                                rhs=qT_h(h)[:, t0 + tq * 512:
                                            t0 + (tq + 1) * 512],
                                start=True, stop=True)
                            if tq == 1:
                                mm.ins.ldweights = False
                        ex = epool.tile([P, THALF], F16, name="ex", tag="ex",
                                        bufs=6)
                        nc.scalar.activation(
                            ex[:], ps[:], mybir.ActivationFunctionType.Exp,
                            bias=0.0, scale=SCALE)
                        exs.append(ex)
                        if j >= 1:
                            pv(j - 1, last=False)
                        if j == 2 and th == 1:
                            # inject a t-half-0 projection chunk while the
                            # scalar engine has exp backlog, instead of at the
                            # head boundary where it would stall the ACT queue
                            proj_chunk(h)
                    pv(nchh - 1, last=True)
                    # normalize: rows 0:64 are o^T, row 64 is denom.
                    # custom-DVE ops can't read PSUM correctly; stage the
                    # denominator row through SBUF first.
                    den = npool.tile([1, THALF], F32, name="den", tag="den")
                    nc.vector.tensor_copy(den[:], po[64:65, :])
                    rec = npool.tile([1, THALF], F32, name="rec", tag="rec")
                    nc.vector.reciprocal_approx_fast(rec[:], den[:])
                    rbc = npool.tile([64, THALF], F32, name="rbc", tag="rbc")
                    nc.gpsimd.partition_broadcast(rbc[:], rec[:])
                    r0 = (h % 2) * 64
                    nc.vector.tensor_mul(
                        oT_sb[h // 2][r0:r0 + 64, t0:t0 + THALF],
                        po[0:64, :], rbc[:])
            for tq in range(8, 16):
                proj_chunk(tq)

    nc.compile()
    return nc


_PROGRAMS = {}


def _get_program(nch4, nch2, nch1):
    key = (nch4, nch2, nch1)
    if key not in _PROGRAMS:
        _PROGRAMS[key] = build_program(*key)
    return _PROGRAMS[key]


def _prep_core_inputs(query, key, value, mask, wT, pb, keeps, nchs):
    nch4, nch2, nch1 = nchs
    N4, N2, N1 = nch4 * P, nch2 * P, nch1 * P
    ins = []
    for b in range(B):
        qb = np.ascontiguousarray(query[:, b, :].T).astype(np.float16)

        def build_k(sub, idx, c0, c1, N):
            # sub: (Ssub, 512) strided keys; gather idx rows, heads c0:c1
            z = np.zeros((P, N), dtype=np.float16)
            g = sub[idx]
            n = len(idx)
            z[:, 0:n] = g[:, c0:c1].T.astype(np.float16)
            return z

        def build_va(sub, idx, heads, W, nch):
            g = sub[idx]
            n = len(idx)
            z = np.zeros((P, nch * W), dtype=np.float16)
            for j in range(nch):
                seg = g[j * P:(j + 1) * P]
                m = seg.shape[0]
                if m == 0:
                    break
                for i, h in enumerate(heads):
                    z[0:m, j * W + i * 65: j * W + i * 65 + 64] = \
                        seg[:, h * 64:(h + 1) * 64].astype(np.float16)
                    z[0:m, j * W + i * 65 + 64] = 1.0
            return z

        kb, vb = key[:, b, :], value[:, b, :]
        i4, i2, i1 = keeps[4][b], keeps[2][b], keeps[1][b]
        ins.append({
            "qT": qb,
            "k4T": build_k(kb[::4], i4, 0, 128, N4),
            "k2T": build_k(kb[::2], i2, 128, 256, N2),
            "k1Ta": build_k(kb, i1, 256, 384, N1),
            "k1Tb": build_k(kb, i1, 384, 512, N1),
            "va4": build_va(vb[::4], i4, [0, 1], 130, nch4),
            "va2": build_va(vb[::2], i2, [2, 3], 130, nch2),
            "va1": build_va(vb, i1, [4, 5, 6, 7], 260, nch1),
            "wT": wT, "pb": pb,
        })
    return ins


def kernel(query, key, value, attn_mask, proj_w, proj_b, _trace=False,
           **run_kwargs):
    query = np.asarray(query, dtype=np.float32)
    key = np.asarray(key, dtype=np.float32)
    value = np.asarray(value, dtype=np.float32)
    mask = np.asarray(attn_mask).astype(bool)
    wT = np.ascontiguousarray(
        np.asarray(proj_w, dtype=np.float32).T).astype(np.float16)
    pb = np.ascontiguousarray(
        np.asarray(proj_b, dtype=np.float32).reshape(1, E))

    keeps = {ks: [np.flatnonzero(~mask[b, ::ks]) for b in range(B)]
             for ks in (4, 2, 1)}
    nchs = tuple(
        max(1, -(-max(len(keeps[ks][b]) for b in range(B)) // P))
        for ks in (4, 2, 1))

    nc = _get_program(*nchs)
    ins = _prep_core_inputs(query, key, value, mask, wT, pb, keeps, nchs)
    res = run_bass_kernel_spmd(nc, ins, list(range(B)), trace=_trace,
                               **run_kwargs)
    outs = [np.asarray(res.results[b]["out"]) for b in range(B)]
    full = np.concatenate(outs, axis=0)          # (B*T, E), b-major rows
    result = full.reshape(T, B, E)
    if _trace:
        return result, res
    return result


# revision 13
# speedup vs baseline: 2.9762x; 1.0632x over previous
"""Sparse multi-head attention (per-head strided K/V subsampling) for trn2.

Problem (hardcoded):
  query/key/value: (2048, 8, 512) f32, attn_mask: (8, 2048) bool,
  proj_w: (512, 512), proj_b: (512,).
  Per head h (8 heads, head_dim 64) with stride ksz in [4,4,2,2,1,1,1,1]:
    scores = q_h @ k_h[::ksz].T * 0.125, masked softmax over subsampled keys,
    o_h = softmax @ v_h[::ksz].
  Reference then does a RAW reshape (B,T,D)->(T,B,D) per head before concat +
  out-projection.  That reshape is a pure row permutation of the flattened
  (B*T, 512) matrix, so computing per-(batch,head) attention in (t, d) layout,
  concatenating per batch, projecting, stacking batches, and reshaping
  (B*T, 512) -> (T, B, 512) reproduces it exactly.

Sharding: batch-parallel, one batch element per NeuronCore (8 cores).

Device/layout design (measured-on-HW rationale):
  - mask-gather on the host: masked keys contribute exactly zero, so only
    unmasked subsampled keys are shipped (~50%). Pad rows are all-zero
    INCLUDING the ones-column of the V-augmented matrix, so pads add 0 to
    both numerator and denominator (their exp(0)=1 hits zero V rows).
  - all matmul operands fp16 (f32r streams ~3x slower per row on real HW).
  - scores computed transposed (s on partitions, t free); V augmented with a
    ones column so one accumulating matmul produces both the unnormalized
    output (rows 0:64 of po) and the softmax denominator (row 64).
  - qT/kT are stored TWICE, with the 64-row halves swapped in the copy: the
    two score matmuls of a chunk then run on opposite PE row groups, so they
    execute concurrently (row tiling) and the second LDWEIGHTS overlaps the
    first matmul instead of serializing.
  - exp fused on ACT: ex = exp(0.125 * scores) in one [128, 1024] ACTIVATE
    per chunk/t-half (ACT is the pacing engine: (N+352)/1.2ns per inst).
  - normalization: po[0:65] is copied to SBUF in one DVE op (releases the
    PSUM accumulator ~1us after the last PV matmul), then
    reciprocal_approx_fast (custom DVE ops misread PSUM, so SBUF source) +
    gpsimd partition-broadcast + DVE multiply, all off the critical path.
  - out-projection with host-side proj_w.T; bias added via DVE tensor_add
    with a pre-broadcast bias tile during the PSUM->SBUF copy. Projection
    chunks for t-half 0 are injected mid-head into the long heads of
    t-half 1, where the ACT queue has backlog to hide them.
  - a short burst of dummy matmuls warms the PE HAM clock gate during the
    initial DMA wait.
"""

import numpy as np

import concourse.bass as bass
import concourse.tile as tile
from concourse import bacc, mybir
from concourse.bass_utils import run_bass_kernel_spmd

T = 2048
B = 8
E = 512
H = 8
D = 64
KS = [4, 4, 2, 2, 1, 1, 1, 1]
SCALE = 0.125
P = 128
THALF = 1024
F32 = mybir.dt.float32
F16 = mybir.dt.float16


def build_program(nch4, nch2, nch1):
    N4, N2, N1 = nch4 * P, nch2 * P, nch1 * P
    nc = bacc.Bacc("TRN2", target_bir_lowering=False, debug=False, num_devices=B)

    qT = nc.dram_tensor("qT", [2 * E, T], F16, kind="ExternalInput")
    k4T = nc.dram_tensor("k4T", [2 * P, N4], F16, kind="ExternalInput")
    k2T = nc.dram_tensor("k2T", [2 * P, N2], F16, kind="ExternalInput")
    k1Ta = nc.dram_tensor("k1Ta", [2 * P, N1], F16, kind="ExternalInput")
    k1Tb = nc.dram_tensor("k1Tb", [2 * P, N1], F16, kind="ExternalInput")
    va4 = nc.dram_tensor("va4", [P, nch4 * 130], F16, kind="ExternalInput")
    va2 = nc.dram_tensor("va2", [P, nch2 * 130], F16, kind="ExternalInput")
    va1 = nc.dram_tensor("va1", [P, nch1 * 260], F16, kind="ExternalInput")
    wT = nc.dram_tensor("wT", [E, E], F16, kind="ExternalInput")
    pb = nc.dram_tensor("pb", [1, E], F32, kind="ExternalInput")
    out = nc.dram_tensor("out", [T, E], F32, kind="ExternalOutput")

    NCH = [nch4, nch4, nch2, nch2, nch1, nch1, nch1, nch1]

    with tile.TileContext(nc) as tc:
        with (
            tc.tile_pool(name="const", bufs=1) as cpool,
            tc.tile_pool(name="exp", bufs=4) as epool,
            tc.tile_pool(name="norm", bufs=2) as npool,
            tc.tile_pool(name="outsb", bufs=4) as opool,
            tc.tile_pool(name="psA", bufs=1, space="PSUM") as pspool,
        ):
            # ---- persistent SBUF loads (ordered by first use) ----
            qTA_sb, qTB_sb = [], []
            for p_ in range(4):
                qTA_sb.append(cpool.tile([P, T], F16, name=f"qTA{p_}",
                                         tag=f"qTA{p_}"))
                qTB_sb.append(cpool.tile([P, T], F16, name=f"qTB{p_}",
                                         tag=f"qTB{p_}"))
            nc.sync.dma_start(qTA_sb[0][:], qT.ap()[0:P, :])
            nc.sync.dma_start(qTB_sb[0][:], qT.ap()[E:E + P, :])

            def kpair(name, dram, N):
                a = cpool.tile([P, N], F16, name=name + "a", tag=name + "a")
                b = cpool.tile([P, N], F16, name=name + "b", tag=name + "b")
                nc.sync.dma_start(a[:], dram.ap()[0:P, :])
                nc.sync.dma_start(b[:], dram.ap()[P:2 * P, :])
                return a, b

            k4A, k4B = kpair("k4", k4T, N4)
            va4_sb = cpool.tile([P, nch4 * 130], F16, name="va4s", tag="va4s")
            nc.sync.dma_start(va4_sb[:], va4.ap())
            k2A, k2B = kpair("k2", k2T, N2)
            nc.sync.dma_start(qTA_sb[1][:], qT.ap()[P:2 * P, :])
            nc.sync.dma_start(qTB_sb[1][:], qT.ap()[E + P:E + 2 * P, :])
            va2_sb = cpool.tile([P, nch2 * 130], F16, name="va2s", tag="va2s")
            nc.sync.dma_start(va2_sb[:], va2.ap())
            k1aA, k1aB = kpair("k1a", k1Ta, N1)
            nc.sync.dma_start(qTA_sb[2][:], qT.ap()[2 * P:3 * P, :])
            nc.sync.dma_start(qTB_sb[2][:], qT.ap()[E + 2 * P:E + 3 * P, :])
            k1bA, k1bB = kpair("k1b", k1Tb, N1)
            nc.sync.dma_start(qTA_sb[3][:], qT.ap()[3 * P:4 * P, :])
            nc.sync.dma_start(qTB_sb[3][:], qT.ap()[E + 3 * P:E + 4 * P, :])
            va1_sb = cpool.tile([P, nch1 * 260], F16, name="va1s", tag="va1s")
            nc.sync.dma_start(va1_sb[:], va1.ap())
            wT_sb = []
            for i in range(4):
                t_ = cpool.tile([P, E], F16, name=f"wT{i}", tag=f"wT{i}")
                nc.sync.dma_start(t_[:], wT.ap()[i * P:(i + 1) * P, :])
                wT_sb.append(t_)
            pb_sb = cpool.tile([1, E], F32, name="pbs", tag="pbs")
            nc.sync.dma_start(pb_sb[:], pb.ap())
            pbb_sb = cpool.tile([P, E], F32, name="pbb", tag="pbb")
            nc.gpsimd.partition_broadcast(pbb_sb[:], pb_sb[:])

            # ---- PE warm-up burst ----
            # The HAM clock gate keeps the PE at 1.2 GHz until it sees
            # sustained matmul activity; fill the initial DMA-wait dead time
            # with back-to-back dummy matmuls (skinny 4-col stationary so
            # LDWEIGHTS stays tiny and PE duty stays high).
            wu_sb = cpool.tile([64, 512], F16, name="wu", tag="wu")
            nc.gpsimd.memset(wu_sb[:], 0.0)
            warm_ps = pspool.tile([P, THALF], F32, name="ps", tag="ps",
                                  bufs=2)
            for i in range(8):
                nc.tensor.matmul(
                    warm_ps[0:4, 0:512], lhsT=wu_sb[:, 0:4], rhs=wu_sb[:],
                    start=True, stop=True)

            # per-head views: (tile, row0) for tq0 (natural) / tq1 (swapped)
            def kT_h(h, tq):
                A, Bt = [(k4A, k4B), (k4A, k4B), (k2A, k2B), (k2A, k2B),
                         (k1aA, k1aB), (k1aA, k1aB), (k1bA, k1bB),
                         (k1bA, k1bB)][h]
                if tq == 0:
                    return A, (h % 2) * 64
                return Bt, (1 - h % 2) * 64

            def qT_h(h, tq):
                if tq == 0:
                    return qTA_sb[h // 2], (h % 2) * 64
                return qTB_sb[h // 2], (1 - h % 2) * 64

            def va_h(h, j):
                if h < 2:
                    return va4_sb[:, j * 130 + h * 65: j * 130 + h * 65 + 65]
                if h < 4:
                    return va2_sb[:, j * 130 + (h - 2) * 65:
                                  j * 130 + (h - 2) * 65 + 65]
                return va1_sb[:, j * 260 + (h - 4) * 65:
                              j * 260 + (h - 4) * 65 + 65]

            # transposed normalized head outputs (fp16), feeding proj
            oT_sb = []
            for p_ in range(4):
                t_ = cpool.tile([P, T], F16, name=f"oT{p_}", tag=f"oT{p_}")
                oT_sb.append(t_)

            def proj_chunk(tq):
                pp_full = pspool.tile([P, THALF], F32, name="pp", tag="ps",
                                      bufs=2)
                pp = pp_full[:, 0:E]
                for i in range(4):
                    nc.tensor.matmul(
                        pp, lhsT=oT_sb[i][:, tq * P:(tq + 1) * P],
                        rhs=wT_sb[i][:], start=(i == 0), stop=(i == 3))
                ot = opool.tile([P, E], F32, name="ot", tag="ot")
                nc.vector.tensor_add(ot[:], pp, pbb_sb[:])
                nc.sync.dma_start(out.ap()[tq * P:(tq + 1) * P, :], ot[:])

            # ---- attention main loop ----
            for th in range(2):
                t0 = th * THALF
                for h in range(H):
                    nchh = NCH[h]
                    po = pspool.tile([P, THALF], F32, name="po", tag="po",
                                     bufs=2)
                    exs = []

                    def pv(j, last):
                        for tq in range(2):
                            nc.tensor.matmul(
                                po[0:65, tq * 512:(tq + 1) * 512],
                                lhsT=va_h(h, j),
                                rhs=exs[j][:, tq * 512:(tq + 1) * 512],
                                start=(j == 0), stop=last)

                    for j in range(nchh):
                        ps = pspool.tile([P, THALF], F32, name="ps", tag="ps",
                                         bufs=2)
                        for tq in range(2):
                            kt, kr = kT_h(h, tq)
                            qt, qr = qT_h(h, tq)
                            nc.tensor.matmul(
                                ps[:, tq * 512:(tq + 1) * 512],
                                lhsT=kt[kr:kr + 64, j * P:(j + 1) * P],
                                rhs=qt[qr:qr + 64, t0 + tq * 512:
                                       t0 + (tq + 1) * 512],
                                start=True, stop=True)
                        ex = epool.tile([P, THALF], F16, name="ex", tag="ex",
                                        bufs=6)
                        nc.scalar.activation(
                            ex[:], ps[:], mybir.ActivationFunctionType.Exp,
                            bias=0.0, scale=SCALE)
                        exs.append(ex)
                        if j >= 1:
                            pv(j - 1, last=False)
                        # inject t-half-0 projection chunks into the long
                        # heads of t-half 1, where ACT backlog hides them
                        if th == 1 and h >= 4 and j in (2, 5):
                            proj_chunk(2 * (h - 4) + (0 if j == 2 else 1))
                    pv(nchh - 1, last=True)
                    # normalize: two DVE copies release po quickly, then
                    # recip / broadcast / multiply run from SBUF off the
                    # fast path. den must be a partition-0 tile: custom DVE
                    # ops (reciprocal_approx_fast) misread inputs whose AP
                    # has a non-zero base partition.
                    oU = npool.tile([64, THALF], F32, name="oU", tag="oU")
                    nc.vector.tensor_copy(oU[:], po[0:64, :])
                    den = npool.tile([1, THALF], F32, name="den", tag="den")
                    nc.vector.tensor_copy(den[:], po[64:65, :])
                    rec = npool.tile([1, THALF], F32, name="rec", tag="rec")
                    nc.vector.reciprocal_approx_fast(rec[:], den[:])
                    rbc = npool.tile([64, THALF], F32, name="rbc", tag="rbc")
                    nc.gpsimd.partition_broadcast(rbc[:], rec[:])
                    r0 = (h % 2) * 64
                    nc.vector.tensor_mul(
                        oT_sb[h // 2][r0:r0 + 64, t0:t0 + THALF],
                        oU[:], rbc[:])
            for tq in range(8, 16):
                proj_chunk(tq)

    nc.compile()
    return nc


_PROGRAMS = {}


def _get_program(nch4, nch2, nch1):
    key = (nch4, nch2, nch1)
    if key not in _PROGRAMS:
        _PROGRAMS[key] = build_program(*key)
    return _PROGRAMS[key]


def _swap_halves(m):
    # [128k, N] -> swap the two 64-row halves within each 128-row block
    blocks = [m[i:i + P] for i in range(0, m.shape[0], P)]
    return np.vstack([np.vstack([b[64:P], b[0:64]]) for b in blocks])


def _prep_core_inputs(query, key, value, mask, wT, pb, keeps, nchs):
    nch4, nch2, nch1 = nchs
    N4, N2, N1 = nch4 * P, nch2 * P, nch1 * P
    ins = []
    for b in range(B):
        qb = np.ascontiguousarray(query[:, b, :].T).astype(np.float16)
        qbd = np.vstack([qb, _swap_halves(qb)])

        def build_k(sub, idx, c0, c1, N):
            z = np.zeros((P, N), dtype=np.float16)
            g = sub[idx]
            n = len(idx)
            z[:, 0:n] = g[:, c0:c1].T.astype(np.float16)
            return np.vstack([z, _swap_halves(z)])

        def build_va(sub, idx, heads, W, nch):
            g = sub[idx]
            z = np.zeros((P, nch * W), dtype=np.float16)
            for j in range(nch):
                seg = g[j * P:(j + 1) * P]
                m = seg.shape[0]
                if m == 0:
                    break
                for i, h in enumerate(heads):
                    z[0:m, j * W + i * 65: j * W + i * 65 + 64] = \
                        seg[:, h * 64:(h + 1) * 64].astype(np.float16)
                    z[0:m, j * W + i * 65 + 64] = 1.0
            return z

        kb, vb = key[:, b, :], value[:, b, :]
        i4, i2, i1 = keeps[4][b], keeps[2][b], keeps[1][b]
        ins.append({
            "qT": qbd,
            "k4T": build_k(kb[::4], i4, 0, 128, N4),
            "k2T": build_k(kb[::2], i2, 128, 256, N2),
            "k1Ta": build_k(kb, i1, 256, 384, N1),
            "k1Tb": build_k(kb, i1, 384, 512, N1),
            "va4": build_va(vb[::4], i4, [0, 1], 130, nch4),
            "va2": build_va(vb[::2], i2, [2, 3], 130, nch2),
            "va1": build_va(vb, i1, [4, 5, 6, 7], 260, nch1),
            "wT": wT, "pb": pb,
        })
    return ins


def kernel(query, key, value, attn_mask, proj_w, proj_b, _trace=False,
           **run_kwargs):
    query = np.asarray(query, dtype=np.float32)
    key = np.asarray(key, dtype=np.float32)
    value = np.asarray(value, dtype=np.float32)
    mask = np.asarray(attn_mask).astype(bool)
    wT = np.ascontiguousarray(
        np.asarray(proj_w, dtype=np.float32).T).astype(np.float16)
    pb = np.ascontiguousarray(
        np.asarray(proj_b, dtype=np.float32).reshape(1, E))

    keeps = {ks: [np.flatnonzero(~mask[b, ::ks]) for b in range(B)]
             for ks in (4, 2, 1)}
    nchs = tuple(
        max(1, -(-max(len(keeps[ks][b]) for b in range(B)) // P))
        for ks in (4, 2, 1))

    nc = _get_program(*nchs)
    ins = _prep_core_inputs(query, key, value, mask, wT, pb, keeps, nchs)
    res = run_bass_kernel_spmd(nc, ins, list(range(B)), trace=_trace,
                               **run_kwargs)
    outs = [np.asarray(res.results[b]["out"]) for b in range(B)]
    full = np.concatenate(outs, axis=0)          # (B*T, E), b-major rows
    result = full.reshape(T, B, E)
    if _trace:
        return result, res
    return result
